# revision 15
# baseline (speedup 1.0000x reference)
"""Trainium2 Bass kernel for nn_DynamicFusionModule.

Pure data-parallel over batch B=8 across 8 NeuronCores (one sample per
core). Everything runs on-device in channel-major layout [C, N]
(C=256 channels over 2 partition chunks, N=1024 tokens on the free
axis):

  - SamplingAgent 1x1 convs in fp32 on the PE (scores feed the top-k
    selection, so they need full precision).
  - Top-k mask via rank counting: rank[j] = #{i: s[i] > s[j]} computed
    with 8 per-partition-scalar compares (exact 0/1 in bf16) + PE
    ones-matmul partition reduction; mask = rank <= max(1024*kr-1, 63).
  - Mixers (pre-LN transformer blocks) with bf16 matmuls (fp32 psum
    accumulation). LayerNorm gamma/beta are folded into the following
    weight matrices on the host; k/v qkv-biases are dropped/folded
    (softmax-invariant / out-proj fold). Attention is computed
    transposed (S[k,q]) so softmax needs no P transposes: the key mask
    is a per-partition bias on the exp, the denominator rides an extra
    ones-column on V, and normalization happens on O.
  - Final scatter: out = mask ? (base+delta)*(1+s*hs) : base, with
    per-token rows broadcast across partitions via K=1 PE matmuls.
"""

import numpy as np

DIM = 256
N = 1024
HID = 512
NH = 4
DH = 64
B = 8

_CACHE = {}


def _install_compat():
    """Environment shims: walrus here accepts at most ONE sync-wait per
    instruction; Tile's kernel-tail drain aggregates many -> split them
    onto single-wait DVE nops. Also make upload_artifacts local-only."""
    import concourse.mybir as mybir
    import concourse.tile as tile
    from concourse.vector_clock import ScopedClock
    from concourse import bass_utils

    if not getattr(tile.TileContext, "_drain_patched", False):

        def _patched(self, tick_clock, wait_clock):
            nc = self.nc
            drain_inst = nc.sync.drain()
            wait_clock.add_sem_waits(
                drain_inst.ins, ScopedClock({None: tick_clock.global_clock})
            )
            si = drain_inst.ins.sync_info
            waits = list(si.on_wait)
            if len(waits) > 1:
                drain_inst.ins.sync_info = mybir.SyncInfo(
                    on_wait=[], on_update=list(si.on_update)
                )
                for i in range(len(waits)):
                    nop = nc.vector.engine_nop()
                    nop.ins.sync_info = mybir.SyncInfo(
                        on_wait=waits[i : i + 1], on_update=[]
                    )
            nc.all_engine_barrier()
            assert self.sems is not None
            popped = nc._tile_sem_poison_stack.pop()
            assert popped is self._sem_poison
            nc.clear_and_free_semaphores(list(self.sems.allocated().values()))
            nc.all_engine_barrier()

        tile.TileContext._drain_and_barrier = _patched
        tile.TileContext._drain_patched = True

    bass_utils.upload_artifacts = lambda tmpdir: str(tmpdir)


def _to_bf16(a):
    import ml_dtypes

    return np.asarray(a, dtype=np.float32).astype(ml_dtypes.bfloat16)


def _prep_inputs(inputs):
    """Host-side packing: per-core activation tensors + replicated
    (layout-transposed, LN-folded) weights."""
    f = {k: np.asarray(v, dtype=np.float32) for k, v in inputs.items()}
    shared = {}

    ascale = (f["bn_g"] / np.float32(np.sqrt(1.0 + 1e-5))).astype(np.float32)
    abias = (f["ab1"] * ascale + f["bn_b"]).astype(np.float32)
    shared["aw1T"] = np.ascontiguousarray(f["aw1"].T).astype(np.float32)
    shared["agsc"] = ascale
    shared["agbi"] = abias
    shared["aw2T"] = np.ascontiguousarray(f["aw2"].reshape(1, HID).T).reshape(HID)
    scal = np.zeros((1, 8), np.float32)
    scal[0, 0] = f["ab2"].reshape(-1)[0]
    scal[0, 1] = f["hb2"].reshape(-1)[0]
    scal[0, 2] = np.float32(f["highlight_scale"])
    scal[0, 3] = np.float32(-0.5)
    scal[0, 4] = np.float32(1e-5)
    shared["scal"] = scal
    hrow = np.zeros((1, 48), np.float32)
    hrow[0, 0:16] = f["hw1"].reshape(16) / np.float32(N)
    hrow[0, 16:32] = f["hb1"].reshape(16)
    hrow[0, 32:48] = f["hw2"].reshape(16)
    shared["hrow"] = hrow

    for p in ("ir", "vis"):
        ln_g = f[p + "_ln_g"]
        ln_b = f[p + "_ln_b"]
        qkv_w = f[p + "_qkv_w"]
        qkv_b = f[p + "_qkv_b"]
        out_w = f[p + "_out_w"]
        out_b = f[p + "_out_b"]
        w1 = f[p + "_ffn_w1"]
        b1 = f[p + "_ffn_b1"]
        w2 = f[p + "_ffn_w2"]
        b2 = f[p + "_ffn_b2"]
        qkvT_eff = (qkv_w * ln_g[None, :]).T  # [256, 768]
        qkvb_eff = qkv_b + qkv_w @ ln_b
        w1T_eff = (w1 * ln_g[None, :]).T  # [256, 1024]
        b1_eff = b1 + w1 @ ln_b
        bv = qkvb_eff[2 * DIM :]
        outb_eff = out_b + out_w @ bv
        shared[p + "_qkvT"] = _to_bf16(qkvT_eff)
        shared[p + "_qb"] = qkvb_eff[:DIM].astype(np.float32)
        shared[p + "_outT"] = _to_bf16(out_w.T)
        shared[p + "_outb"] = outb_eff.astype(np.float32)
        shared[p + "_w1T"] = _to_bf16(w1T_eff)
        shared[p + "_b1"] = b1_eff.astype(np.float32)
        shared[p + "_w2T"] = _to_bf16(w2.T)
        shared[p + "_b2"] = b2.astype(np.float32)

    per_core = []
    fir = f["f_ir"].reshape(B, DIM, N)
    fvis = f["f_vis"].reshape(B, DIM, N)
    for b in range(B):
        m = dict(shared)
        m["fir"] = np.ascontiguousarray(fir[b])
        m["fvis"] = np.ascontiguousarray(fvis[b])
        per_core.append(m)
    return per_core




def _split_multi_waits(nc):
    """This container's walrus accepts only ONE sync-wait per
    instruction: hoist extra waits onto same-engine nop carriers
    inserted immediately before the instruction."""
    import concourse.mybir as mybir

    for f in nc.m.functions:
        for bb in f.blocks:
            insts = list(bb.instructions)
            rebuilt = []
            changed = False
            for inst in insts:
                si = inst.sync_info
                waits = list(si.on_wait) if si is not None else []
                if len(waits) > 1:
                    changed = True
                    eng = inst.engine
                    for wx in waits[:-1]:
                        wrap = nc.engines[eng].nop(nofuse=True)
                        mi = wrap.ins
                        # remove from wherever add_instruction appended it
                        for f2 in nc.m.functions:
                            for bb2 in f2.blocks:
                                lst = list(bb2.instructions)
                                if lst and lst[-1] is mi:
                                    lst.pop()
                                    bb2.instructions = lst
                        mi.sync_info = mybir.SyncInfo(on_wait=[wx], on_update=[])
                        rebuilt.append(mi)
                    inst.sync_info = mybir.SyncInfo(
                        on_wait=[waits[-1]], on_update=list(si.on_update)
                    )
                rebuilt.append(inst)
            if changed:
                bb.instructions = rebuilt


def _build(debug=False):
    from contextlib import ExitStack

    import concourse.bass as bass
    import concourse.mybir as mybir
    import concourse.tile as tile
    from concourse.masks import make_identity

    f32 = mybir.dt.float32
    bf16 = mybir.dt.bfloat16
    AF = mybir.ActivationFunctionType
    OP = mybir.AluOpType
    AX = mybir.AxisListType

    nc = bass.Bass(
        "TRN2", target_bir_lowering=False, debug=False, enable_asserts=True
    )

    def din(name, shape, dt=f32):
        return nc.dram_tensor(name, shape, dt, kind="ExternalInput").ap()

    fir_d = din("fir", [DIM, N])
    fvis_d = din("fvis", [DIM, N])
    aw1T_d = din("aw1T", [2 * DIM, HID])
    agsc_d = din("agsc", [HID])
    agbi_d = din("agbi", [HID])
    aw2T_d = din("aw2T", [HID])
    scal_d = din("scal", [1, 8])
    hrow_d = din("hrow", [1, 48])
    mix_d = {}
    for p in ("ir", "vis"):
        mix_d[p] = dict(
            qkvT=din(p + "_qkvT", [DIM, 3 * DIM], bf16),
            qb=din(p + "_qb", [DIM]),
            outT=din(p + "_outT", [DIM, DIM], bf16),
            outb=din(p + "_outb", [DIM]),
            w1T=din(p + "_w1T", [DIM, 4 * DIM], bf16),
            b1=din(p + "_b1", [4 * DIM]),
            w2T=din(p + "_w2T", [4 * DIM, DIM], bf16),
            b2=din(p + "_b2", [DIM]),
        )

    out_d = nc.dram_tensor("out", [DIM, N], f32, kind="ExternalOutput").ap()
    dbg_d = {}
    if debug:
        for nm, shp in (
            ("d_w", [1, N]),
            ("d_s", [1, N]),
            ("d_rank", [1, N]),
            ("d_mask", [1, N]),
            ("d_thr", [1, 8]),
            ("d_refir", [DIM, N]),
            ("d_refvis", [DIM, N]),
        ):
            dbg_d[nm] = nc.dram_tensor(nm, shp, f32, kind="ExternalOutput").ap()

    ctx = ExitStack()
    with tile.TileContext(nc) as tc:
        wp = ctx.enter_context(tc.tile_pool(name="wp", bufs=1))
        xp = ctx.enter_context(tc.tile_pool(name="xp", bufs=1))
        hp = ctx.enter_context(tc.tile_pool(name="hp", bufs=1))
        sp = ctx.enter_context(tc.tile_pool(name="sp", bufs=2))
        # PSUM: 16KB/partition total.  sps 2x4KB + ops 4KB + misc 4KB.
        pps = ctx.enter_context(tc.tile_pool(name="pps", bufs=2, space="PSUM"))
        ppo = ctx.enter_context(tc.tile_pool(name="ppo", bufs=1, space="PSUM"))
        ppm = ctx.enter_context(tc.tile_pool(name="ppm", bufs=1, space="PSUM"))

        def col128(dram_vec, n):
            t = wp.tile([128, n], f32, tag=dram_vec.tensor.name)
            nc.sync.dma_start(t[:], dram_vec.rearrange("(c p) -> p c", p=128))
            return t

        # ---- weights ----
        aw1T_sb = []
        for kc in range(4):
            t = wp.tile([128, HID], f32, tag=f"aw1T{kc}")
            nc.sync.dma_start(t[:], aw1T_d[kc * 128 : (kc + 1) * 128, :])
            aw1T_sb.append(t)
        agsc_sb = col128(agsc_d, 4)
        agbi_sb = col128(agbi_d, 4)
        aw2c_sb = col128(aw2T_d, 4)
        scal_sb = wp.tile([1, 8], f32, tag="scal")
        nc.sync.dma_start(scal_sb[:], scal_d[:])
        hrow_sb = wp.tile([1, 48], f32, tag="hrow")
        nc.sync.dma_start(hrow_sb[:], hrow_d[:])

        mix_w = {}
        for p in ("ir", "vis"):
            d = mix_d[p]
            w = {"qkvT": [], "outT": [], "w1T": [], "w2T": []}
            for c in range(2):
                t = wp.tile([128, 3 * DIM], bf16, tag=f"{p}qkvT{c}")
                nc.sync.dma_start(t[:], d["qkvT"][c * 128 : (c + 1) * 128, :])
                w["qkvT"].append(t)
            w["qb"] = col128(d["qb"], 2)
            for c in range(2):
                t = wp.tile([128, DIM], bf16, tag=f"{p}outT{c}")
                nc.sync.dma_start(t[:], d["outT"][c * 128 : (c + 1) * 128, :])
                w["outT"].append(t)
            w["outb"] = col128(d["outb"], 2)
            for c in range(2):
                t = wp.tile([128, 4 * DIM], bf16, tag=f"{p}w1T{c}")
                nc.sync.dma_start(t[:], d["w1T"][c * 128 : (c + 1) * 128, :])
                w["w1T"].append(t)
            w["b1"] = col128(d["b1"], 8)
            for kc in range(8):
                t = wp.tile([128, DIM], bf16, tag=f"{p}w2T{kc}")
                nc.sync.dma_start(t[:], d["w2T"][kc * 128 : (kc + 1) * 128, :])
                w["w2T"].append(t)
            w["b2"] = col128(d["b2"], 2)
            mix_w[p] = w

        ones_row = wp.tile([1, 128], f32, tag="ones_row")
        nc.vector.memset(ones_row[:], 1.0)
        ones_colb = wp.tile([128, 1], bf16, tag="ones_colb")
        nc.vector.memset(ones_colb[:], 1.0)
        ident = wp.tile([128, 128], f32, tag="ident")
        make_identity(nc, ident[:])
        ident_bf = wp.tile([64, 64], bf16, tag="ident_bf")
        make_identity(nc, ident_bf[:])
        ident_bf_hi = wp.tile([128, 64], bf16, tag="ident_bf_hi")
        nc.gpsimd.memset(ident_bf_hi[:], 0.0)
        make_identity(nc, ident_bf_hi[64:128, 0:64], nomemset=True)

        # ---- activations in ----
        X = {}
        fir_sb, fvis_sb, base_sb = [], [], []
        for c in range(2):
            t = xp.tile([128, N], f32, tag=f"fir{c}")
            nc.sync.dma_start(t[:], fir_d[c * 128 : (c + 1) * 128, :])
            fir_sb.append(t)
            t = xp.tile([128, N], f32, tag=f"fvis{c}")
            nc.sync.dma_start(t[:], fvis_d[c * 128 : (c + 1) * 128, :])
            fvis_sb.append(t)
        for c in range(2):
            t = xp.tile([128, N], f32, tag=f"base{c}")
            nc.gpsimd.tensor_tensor(t[:], fir_sb[c][:], fvis_sb[c][:], op=OP.add)
            base_sb.append(t)
        X["ir"] = fir_sb
        X["vis"] = fvis_sb
        xcat = fir_sb + fvis_sb

        # =========== agent (fp32) ===========
        h_sb = []
        for oc in range(4):
            ps = pps.tile([128, N], f32, tag="sps")
            for half in range(2):
                hs = slice(half * 512, half * 512 + 512)
                for kc in range(4):
                    nc.tensor.matmul(
                        ps[:, hs],
                        aw1T_sb[kc][:, oc * 128 : (oc + 1) * 128],
                        xcat[kc][:, hs],
                        start=(kc == 0),
                        stop=(kc == 3),
                    )
            t = xp.tile([128, N], f32, tag=f"h{oc}")
            nc.scalar.activation(
                t[:],
                ps[:],
                AF.Relu,
                bias=agbi_sb[:, oc : oc + 1],
                scale=agsc_sb[:, oc : oc + 1],
            )
            h_sb.append(t)
        zps = ppm.tile([1, N], f32, tag="misc")
        for half in range(2):
            hs = slice(half * 512, half * 512 + 512)
            for oc in range(4):
                nc.tensor.matmul(
                    zps[0:1, hs],
                    aw2c_sb[:, oc : oc + 1],
                    h_sb[oc][:, hs],
                    start=(oc == 0),
                    stop=(oc == 3),
                )
        w_row = xp.tile([1, N], f32, tag="w_row")
        nc.scalar.activation(
            w_row[:], zps[:], AF.Sigmoid, bias=scal_sb[0:1, 0:1], scale=1.0
        )
        s_row = xp.tile([1, N], f32, tag="s_row")
        nc.scalar.activation(s_row[:], w_row[:], AF.Abs, bias=scal_sb[0:1, 3:4], scale=1.0)
        e_row = xp.tile([1, N], f32, tag="e_row")
        nc.vector.tensor_scalar(
            e_row[:], s_row[:], scal_sb[0:1, 2:3], 1.0, op0=OP.mult, op1=OP.add
        )

        # ---- k-ratio MLP -> rank threshold ----
        gsum = sp.tile([1, 1], f32, tag="tiny")
        nc.vector.reduce_sum(gsum[:], s_row[:], axis=AX.X)
        t16 = sp.tile([1, 16], f32, tag="t16")
        nc.vector.tensor_scalar(
            t16[:], hrow_sb[0:1, 0:16], gsum[0:1, 0:1], None, op0=OP.mult
        )
        nc.vector.tensor_tensor(t16[:], t16[:], hrow_sb[0:1, 16:32], op=OP.add)
        t16r = sp.tile([1, 16], f32, tag="t16r")
        nc.scalar.activation(t16r[:], t16[:], AF.Relu)
        t16j = sp.tile([1, 16], f32, tag="t16j")
        urow = sp.tile([1, 1], f32, tag="tiny2")
        nc.vector.scalar_tensor_tensor(
            t16j[:],
            t16r[:],
            1.0,
            hrow_sb[0:1, 32:48],
            op0=OP.mult,
            op1=OP.mult,
            accum_out=urow[:],
        )
        thr = xp.tile([1, 1], f32, tag="thr")
        nc.scalar.activation(
            thr[:], urow[:], AF.Sigmoid, bias=scal_sb[0:1, 1:2], scale=1.0
        )
        nc.vector.tensor_scalar(
            thr[:], thr[:], 819.2, 102.4 - 1.0, op0=OP.mult, op1=OP.add
        )
        nc.vector.tensor_scalar(thr[:], thr[:], 63.0, None, op0=OP.max)

        # ---- ranks via counting compares ----
        s_col = xp.tile([128, 8], f32, tag="s_col")
        for t in range(8):
            tp = ppm.tile([128, 64], f32, tag="misc")
            nc.tensor.transpose(
                tp[:, 0:1], s_row[0:1, t * 128 : (t + 1) * 128], ident[0:1, 0:1]
            )
            nc.vector.tensor_copy(s_col[:, t : t + 1], tp[:, 0:1])
        sbc_ps = pps.tile([128, N], f32, tag="sps")
        for half in range(2):
            hs = slice(half * 512, half * 512 + 512)
            nc.tensor.matmul(
                sbc_ps[:, hs], ones_row[:], s_row[0:1, hs], start=True, stop=True
            )
        s_bc = xp.tile([128, N], f32, tag="s_bc")
        nc.scalar.copy(s_bc[:], sbc_ps[:])
        rank_ps = ppm.tile([1, N], f32, tag="misc")
        for t in range(8):
            cmp = sp.tile([128, N], bf16, tag="cmp")
            nc.vector.tensor_scalar(
                cmp[:], s_bc[:], s_col[:, t : t + 1], None, op0=OP.is_lt
            )
            for half in range(2):
                hs = slice(half * 512, half * 512 + 512)
                nc.tensor.matmul(
                    rank_ps[0:1, hs],
                    ones_colb[:],
                    cmp[:, hs],
                    start=(t == 0),
                    stop=(t == 7),
                )
        mask_row = xp.tile([1, N], f32, tag="mask_row")
        nc.vector.tensor_scalar(
            mask_row[:], rank_ps[:], thr[0:1, 0:1], None, op0=OP.is_le
        )
        mask_col = xp.tile([128, 8], f32, tag="mask_col")
        for t in range(8):
            tp = ppm.tile([128, 64], f32, tag="misc")
            nc.tensor.transpose(
                tp[:, 0:1], mask_row[0:1, t * 128 : (t + 1) * 128], ident[0:1, 0:1]
            )
            nc.vector.tensor_copy(mask_col[:, t : t + 1], tp[:, 0:1])
        mbias_col = xp.tile([128, 8], f32, tag="mbias_col")
        nc.vector.tensor_scalar(
            mbias_col[:], mask_col[:], 1.0, 1e9, op0=OP.subtract, op1=OP.mult
        )

        if debug:
            nc.sync.dma_start(dbg_d["d_w"][:], w_row[:])
            nc.sync.dma_start(dbg_d["d_s"][:], s_row[:])
            dbg_rank = sp.tile([1, N], f32, tag="dbgrank")
            nc.vector.tensor_copy(dbg_rank[:], rank_ps[:])
            nc.sync.dma_start(dbg_d["d_rank"][:], dbg_rank[:])
            nc.sync.dma_start(dbg_d["d_mask"][:], mask_row[:])
            dbg_thr = sp.tile([1, 8], f32, tag="dbgthr")
            nc.vector.memset(dbg_thr[:], 0.0)
            nc.vector.tensor_copy(dbg_thr[:, 0:1], thr[:])
            nc.sync.dma_start(dbg_d["d_thr"][:], dbg_thr[:])

        # =========== mixers ===========
        def layernorm_bf(Xc, stage):
            xbf = []
            for c in range(2):
                t = sp.tile([128, N], bf16, tag="xbf")
                nc.vector.tensor_copy(t[:], Xc[c][:])
                xbf.append(t)
            xsq = []
            for c in range(2):
                t = sp.tile([128, N], bf16, tag="xsq")
                nc.gpsimd.tensor_tensor(t[:], xbf[c][:], xbf[c][:], op=OP.mult)
                xsq.append(t)
            st = ppm.tile([65, N], f32, tag="misc")
            for half in range(2):
                hs = slice(half * 512, half * 512 + 512)
                for c in range(2):
                    nc.tensor.matmul(
                        st[0:1, hs],
                        ones_colb[:],
                        xbf[c][:, hs],
                        start=(c == 0),
                        stop=(c == 1),
                    )
                for c in range(2):
                    nc.tensor.matmul(
                        st[64:65, hs],
                        ones_colb[:],
                        xsq[c][:, hs],
                        start=(c == 0),
                        stop=(c == 1),
                    )
            mrow = sp.tile([1, N], f32, tag="rowf", bufs=3, name="mrow")
            nc.scalar.mul(mrow[:], st[0:1, :], 1.0 / DIM)
            m2row = sp.tile([1, N], f32, tag="rowf", bufs=3, name="m2row")
            nc.scalar.mul(m2row[:], st[64:65, :], 1.0 / DIM)
            vrow = sp.tile([1, N], f32, tag="rowf", bufs=3, name="vrow")
            nc.vector.tensor_tensor(vrow[:], mrow[:], mrow[:], op=OP.mult)
            nc.vector.tensor_tensor(vrow[:], m2row[:], vrow[:], op=OP.subtract)
            sdrow = sp.tile([1, N], f32, tag="rowf", bufs=3, name="sdrow")
            nc.scalar.activation(sdrow[:], vrow[:], AF.Sqrt, bias=scal_sb[0:1, 4:5], scale=1.0)
            irow = sp.tile([1, N], f32, tag="rowf", bufs=3, name="irow")
            nc.vector.reciprocal(irow[:], sdrow[:])
            mb_ps = pps.tile([128, N], f32, tag="sps")
            ib_ps = pps.tile([128, N], f32, tag="sps")
            for half in range(2):
                hs = slice(half * 512, half * 512 + 512)
                nc.tensor.matmul(
                    mb_ps[:, hs], ones_row[:], mrow[0:1, hs], start=True, stop=True
                )
                nc.tensor.matmul(
                    ib_ps[:, hs], ones_row[:], irow[0:1, hs], start=True, stop=True
                )
            mb = sp.tile([128, N], bf16, tag="lnb")
            nc.scalar.copy(mb[:], mb_ps[:])
            ib = sp.tile([128, N], bf16, tag="lnb")
            nc.scalar.copy(ib[:], ib_ps[:])
            xln = []
            for c in range(2):
                t = hp.tile([128, N], bf16, tag=f"xln{stage}{c}")
                nc.vector.tensor_tensor(t[:], xbf[c][:], mb[:], op=OP.subtract)
                nc.vector.tensor_tensor(t[:], t[:], ib[:], op=OP.mult)
                xln.append(t)
            return xln

        def mixer(p):
            w = mix_w[p]
            Xc = X[p]
            xln = layernorm_bf(Xc, "a")
            qkv = []
            for oc in range(6):
                ps = pps.tile([128, N], f32, tag="sps")
                for half in range(2):
                    hs = slice(half * 512, half * 512 + 512)
                    for c in range(2):
                        nc.tensor.matmul(
                            ps[:, hs],
                            w["qkvT"][c][:, oc * 128 : (oc + 1) * 128],
                            xln[c][:, hs],
                            start=(c == 0),
                            stop=(c == 1),
                        )
                t = hp.tile([128, N], bf16, tag=f"qkv{oc}")
                if oc < 2:
                    nc.scalar.activation(
                        t[:], ps[:], AF.Identity, bias=w["qb"][:, oc : oc + 1]
                    )
                else:
                    nc.scalar.copy(t[:], ps[:])
                qkv.append(t)

            o_sb = [
                hp.tile([128, N], bf16, tag="osb0", name="osb0"),
                hp.tile([128, N], bf16, tag="osb1", name="osb1"),
            ]
            for h in range(NH):
                hi = h % 2
                qt = qkv[h // 2][hi * 64 : hi * 64 + 64, :]
                kt = qkv[2 + h // 2][hi * 64 : hi * 64 + 64, :]
                vt = qkv[4 + h // 2][hi * 64 : hi * 64 + 64, :]
                ops = ppo.tile([65, N], f32, tag="ops")
                for kc in range(8):
                    tp = ppm.tile([128, 64], bf16, tag="misc")
                    idnt = ident_bf[:] if hi == 0 else ident_bf_hi[64:128, 0:64]
                    nc.tensor.transpose(
                        tp[:], vt[:, kc * 128 : (kc + 1) * 128], idnt
                    )
                    vtm = sp.tile([128, 65], bf16, tag="vtm")
                    nc.vector.tensor_copy(vtm[:, 0:64], tp[:])
                    nc.vector.memset(vtm[:, 64:65], 1.0)
                    sps = pps.tile([128, N], f32, tag="sps")
                    for half in range(2):
                        hs = slice(half * 512, half * 512 + 512)
                        nc.tensor.matmul(
                            sps[:, hs],
                            kt[:, kc * 128 : (kc + 1) * 128],
                            qt[:, hs],
                            start=True,
                            stop=True,
                        )
                    pt = sp.tile([128, N], bf16, tag="pt")
                    nc.scalar.activation(
                        pt[:],
                        sps[:],
                        AF.Exp,
                        bias=mbias_col[:, kc : kc + 1],
                        scale=0.125,
                    )
                    for half in range(2):
                        hs = slice(half * 512, half * 512 + 512)
                        nc.tensor.matmul(
                            ops[:, hs],
                            vtm[:],
                            pt[:, hs],
                            start=(kc == 0),
                            stop=(kc == 7),
                        )
                rrow = sp.tile([1, N], f32, tag="rowf", bufs=3, name="rrow")
                nc.vector.reciprocal(rrow[:], ops[64:65, :])
                rb_ps = ppm.tile([64, N], f32, tag="misc")
                for half in range(2):
                    hs = slice(half * 512, half * 512 + 512)
                    nc.tensor.matmul(
                        rb_ps[:, hs],
                        ones_row[0:1, 0:64],
                        rrow[0:1, hs],
                        start=True,
                        stop=True,
                    )
                rb = sp.tile([64, N], bf16, tag="rbs")
                nc.scalar.copy(rb[:], rb_ps[:])
                dst = o_sb[h // 2][hi * 64 : hi * 64 + 64, :]
                nc.vector.tensor_tensor(dst, ops[0:64, :], rb[:], op=OP.mult)

            for oc in range(2):
                ps = pps.tile([128, N], f32, tag="sps")
                for half in range(2):
                    hs = slice(half * 512, half * 512 + 512)
                    for c in range(2):
                        nc.tensor.matmul(
                            ps[:, hs],
                            w["outT"][c][:, oc * 128 : (oc + 1) * 128],
                            o_sb[c][:, hs],
                            start=(c == 0),
                            stop=(c == 1),
                        )
                nc.vector.scalar_tensor_tensor(
                    Xc[oc][:],
                    ps[:],
                    w["outb"][:, oc : oc + 1],
                    Xc[oc][:],
                    op0=OP.add,
                    op1=OP.add,
                )

            xln2 = layernorm_bf(Xc, "b")
            hf = []
            for oc in range(8):
                ps = pps.tile([128, N], f32, tag="sps")
                for half in range(2):
                    hs = slice(half * 512, half * 512 + 512)
                    for c in range(2):
                        nc.tensor.matmul(
                            ps[:, hs],
                            w["w1T"][c][:, oc * 128 : (oc + 1) * 128],
                            xln2[c][:, hs],
                            start=(c == 0),
                            stop=(c == 1),
                        )
                t = hp.tile([128, N], bf16, tag=f"hf{oc}")
                nc.scalar.activation(
                    t[:], ps[:], AF.Gelu, bias=w["b1"][:, oc : oc + 1], scale=1.0
                )
                hf.append(t)
            for oc in range(2):
                ps = pps.tile([128, N], f32, tag="sps")
                for half in range(2):
                    hs = slice(half * 512, half * 512 + 512)
                    for kc in range(8):
                        nc.tensor.matmul(
                            ps[:, hs],
                            w["w2T"][kc][:, oc * 128 : (oc + 1) * 128],
                            hf[kc][:, hs],
                            start=(kc == 0),
                            stop=(kc == 7),
                        )
                nc.vector.scalar_tensor_tensor(
                    Xc[oc][:],
                    ps[:],
                    w["b2"][:, oc : oc + 1],
                    Xc[oc][:],
                    op0=OP.add,
                    op1=OP.add,
                )

        mixer("ir")
        mixer("vis")
        if debug:
            for c in range(2):
                nc.sync.dma_start(
                    dbg_d["d_refir"][c * 128 : (c + 1) * 128, :], X["ir"][c][:]
                )
                nc.sync.dma_start(
                    dbg_d["d_refvis"][c * 128 : (c + 1) * 128, :], X["vis"][c][:]
                )

        # =========== final scatter ===========
        bcast = {}
        for nm, rowt in (("w", w_row), ("e", e_row), ("m", mask_row)):
            ps = pps.tile([128, N], f32, tag="sps")
            for half in range(2):
                hs = slice(half * 512, half * 512 + 512)
                nc.tensor.matmul(
                    ps[:, hs], ones_row[:], rowt[0:1, hs], start=True, stop=True
                )
            if nm == "m":
                t = xp.tile([128, N], mybir.dt.uint8, tag="bcm", name="bcm")
                nc.vector.tensor_scalar(t[:], ps[:], 0.5, None, op0=OP.is_gt)
            else:
                t = xp.tile([128, N], f32, tag=f"bc{nm}", name=f"bc{nm}")
                nc.scalar.copy(t[:], ps[:])
            bcast[nm] = t

        for c in range(2):
            d = sp.tile([128, N], f32, tag="fin")
            nc.gpsimd.tensor_tensor(
                d[:], X["ir"][c][:], X["vis"][c][:], op=OP.subtract
            )
            nc.vector.tensor_tensor(d[:], d[:], bcast["w"][:], op=OP.mult)
            nc.gpsimd.tensor_tensor(d[:], d[:], X["vis"][c][:], op=OP.add)
            nc.vector.tensor_tensor(d[:], d[:], base_sb[c][:], op=OP.add)
            nc.vector.tensor_tensor(d[:], d[:], bcast["e"][:], op=OP.mult)
            o = sp.tile([128, N], f32, tag="fin")
            nc.vector.tensor_copy(o[:], base_sb[c][:])
            nc.vector.copy_predicated(o[:], bcast["m"][:], d[:])
            nc.sync.dma_start(out_d[c * 128 : (c + 1) * 128, :], o[:])

        ctx.close()
    _split_multi_waits(nc)
    return nc


def _get_nc(debug=False):
    key = ("nc", debug)
    if key not in _CACHE:
        _install_compat()
        _CACHE[key] = _build(debug=debug)
    return _CACHE[key]


def kernel(**inputs):
    from concourse.bass_utils import run_bass_kernel_spmd

    nc = _get_nc(debug=False)
    per_core = _prep_inputs(inputs)
    res = run_bass_kernel_spmd(nc, per_core, core_ids=list(range(B)))
    out = np.stack([res.results[b]["out"] for b in range(B)], axis=0)
    return out.reshape(B, DIM, 32, 32).astype(np.float32)


def kernel_debug(**inputs):
    from concourse.bass_utils import run_bass_kernel_spmd

    nc = _get_nc(debug=True)
    per_core = _prep_inputs(inputs)
    res = run_bass_kernel_spmd(nc, per_core, core_ids=list(range(B)))
    return res
